# revision 37
# baseline (speedup 1.0000x reference)
"""Multi-head causal attention (B=2, S=2048, H=16, D=64) on 8 TRN2 NeuronCores.

Sharding: data-parallel over batch (2) x tensor-parallel over head groups (4).
Core c handles batch b = c // 4, head group g = c % 4 (heads 4g..4g+3).
Each core computes q/k/v projections for its 4 heads, RoPE, causal
flash-style attention (upper-triangular blocks skipped), and a partial
output projection out_partial = attn_out @ Wo[256g:256g+256].  The host
sums the 4 partials per batch and adds the (bias) terms.

Key layout/precision choices:
 - q/k/v projections run in fp8 DoubleRow mode (0.5 PE cycles/row, two
   128-row k-tiles per instruction).  Accuracy is preserved with a
   hi/lo split: x = x_hi(e4m3) + x_lo(e5m2), W = W_hi + W_lo, and the
   product is the 3-term expansion x_hi*W_hi + x_hi*W_lo + x_lo*W_hi
   accumulated in one PSUM group (12 DR matmuls per 1024-deep chain,
   vs 8 full-rate bf16 matmuls) -- 25% fewer PE cycles than bf16 at
   slightly BETTER precision (lo term recovers the e4m3 rounding).
 - q/k are computed TRANSPOSED (d on partitions); Wq/Wk columns are
   permuted to [all even comps | all odd comps] so RoPE runs as
   full-128-partition DVE ops; a regroup pass makes them head-contiguous.
 - scores are computed transposed (kv on partitions, q free) in bf16.
 - exp(scores) goes to per-quarter persistent SBUF tiles (one per head
   pair).  Causal masking is a post-exp multiply of the diagonal blocks
   by a 0/1 triangle on gpsimd.
 - PV uses exp(scores) as the STATIONARY operand ([kv, q-subtile]) and
   v (plus an appended ones column) as the 65-wide MOVING operand:
   the PE charge is 65 cycles per accumulation step instead of up to
   512 -- half the PV cost of the scores-as-moving mapping.  The ones
   column makes psum col 64 the softmax denominator, per q-partition,
   so normalization is one tiny reciprocal + per-partition
   tensor_scalar multiplies.
 - the normalized [q, hd] tile is transposed back to [hd, q] with a
   128x128 PE transpose against an identity (128 cycles) and copied to
   the bf16 attn-out tile consumed by the out-projection.
 - out-projections (bf16 o x bf16 Wo) are deferred into quarter 3 and
   the epilogue, where ACT is exp-saturated / idle and the proj psum
   pool is free (PSUM budget: proj 2 + scores 4 + pv/pt 2 = 8 banks).
"""

import os
import numpy as np
from contextlib import ExitStack

import concourse.bass as bass
import concourse.tile as tile
from concourse import bacc, mybir
from concourse.bass_utils import run_bass_kernel_spmd

F32 = mybir.dt.float32
BF16 = mybir.dt.bfloat16
FP8H = mybir.dt.float8e4
FP8L = mybir.dt.float8e5
AF = mybir.ActivationFunctionType
DR = mybir.MatmulPerfMode.DoubleRow

B, S, H, D = 2, 2048, 16, 64
HID = H * D           # 1024
NCORES = 8
G = 4                 # head groups
HPG = H // G          # heads per group = 4
DG = HPG * D          # per-group model dim = 256
KS = HID // 128       # 8 k-subtiles
NQ = 4                # S quarters (chunks of 512)
SB = S // 128         # 16 s-blocks


def build_program():
    nc = bacc.Bacc("TRN2", target_bir_lowering=False, debug=False,
                   num_devices=NCORES)

    xh = nc.dram_tensor("xh", [HID, S], FP8H, kind="ExternalInput").ap()
    xl = nc.dram_tensor("xl", [HID, S], FP8L, kind="ExternalInput").ap()
    wqh = nc.dram_tensor("wqh", [HID, DG], FP8H, kind="ExternalInput").ap()
    wql = nc.dram_tensor("wql", [HID, DG], FP8L, kind="ExternalInput").ap()
    wkh = nc.dram_tensor("wkh", [HID, DG], FP8H, kind="ExternalInput").ap()
    wkl = nc.dram_tensor("wkl", [HID, DG], FP8L, kind="ExternalInput").ap()
    wvh = nc.dram_tensor("wvh", [HID, DG], FP8H, kind="ExternalInput").ap()
    wvl = nc.dram_tensor("wvl", [HID, DG], FP8L, kind="ExternalInput").ap()
    wo = nc.dram_tensor("wo", [DG, HID], BF16, kind="ExternalInput").ap()
    bqp = nc.dram_tensor("bqp", [128, 2], F32, kind="ExternalInput").ap()
    bkp = nc.dram_tensor("bkp", [128, 2], F32, kind="ExternalInput").ap()
    cos4 = nc.dram_tensor("cos4", [128, S], BF16, kind="ExternalInput").ap()
    sin4 = nc.dram_tensor("sin4", [128, S], BF16, kind="ExternalInput").ap()
    trid = nc.dram_tensor("trid", [128, 2, 128], BF16, kind="ExternalInput").ap()
    idd = nc.dram_tensor("idd", [128, 128], BF16, kind="ExternalInput").ap()
    out = nc.dram_tensor("out", [S, HID], BF16, kind="ExternalOutput").ap()

    with tile.TileContext(nc) as tc, ExitStack() as ctx:
        const = ctx.enter_context(tc.tile_pool(name="const", bufs=1))
        xp = ctx.enter_context(tc.tile_pool(name="xp", bufs=2))
        tmp = ctx.enter_context(tc.tile_pool(name="tmp", bufs=6))
        stg = ctx.enter_context(tc.tile_pool(name="stg", bufs=4))
        nrm = ctx.enter_context(tc.tile_pool(name="nrm", bufs=2))
        ps = ctx.enter_context(tc.tile_pool(name="ps", bufs=2, space="PSUM"))
        psc = ctx.enter_context(tc.tile_pool(name="psc", bufs=2, space="PSUM"))
        ppv = ctx.enter_context(tc.tile_pool(name="ppv", bufs=2, space="PSUM"))

        # ---- persistent SBUF tiles (DMAs emitted at first-use points) ----
        wqh_t = const.tile([128, KS, DG], FP8H)
        wql_t = const.tile([128, KS, DG], FP8L)
        wkh_t = const.tile([128, KS, DG], FP8H)
        wkl_t = const.tile([128, KS, DG], FP8L)
        wvh_t = const.tile([128, KS, DG], FP8H)
        wvl_t = const.tile([128, KS, DG], FP8L)
        wo_t = const.tile([128, 2, HID], BF16)
        cos_t = const.tile([128, S], BF16)
        sin_t = const.tile([128, S], BF16)
        tri_t = const.tile([128, 2, 128], BF16)
        id_t = const.tile([128, 128], BF16)
        bq_t = const.tile([128, 2], F32)
        bk_t = const.tile([128, 2], F32)
        v1_t = const.tile([128, SB, HPG, D + 1], BF16)   # v blocks + ones col
        qr_t = const.tile([128, 2, S], BF16)   # roped q, [evens|odds] chunks
        kr_t = const.tile([128, 2, S], BF16)
        qh_t = const.tile([128, 2, S], BF16)   # head-contiguous roped q
        kh_t = const.tile([128, 2, S], BF16)
        o_t = const.tile([128, 2, S], BF16)    # attn outT (hd on partitions)
        # exp(scores) tiles [kv-block, head-in-pair, q-col]; quarter j head
        # pair cc uses tile (2j+cc) % 3 -- the 3-way rotation lets pv of the
        # second head pair defer into the NEXT quarter's window without a
        # write-after-read race against that quarter's exps.
        et0_t = const.tile([128, SB, 2, 512], BF16)
        et1_t = const.tile([128, SB, 2, 512], BF16)
        et2_t = const.tile([128, SB, 2, 512], BF16)
        et_tiles = [et0_t, et1_t, et2_t]

        wqhr = wqh.rearrange("(o p) n -> p o n", p=128)
        wqlr = wql.rearrange("(o p) n -> p o n", p=128)
        wkhr = wkh.rearrange("(o p) n -> p o n", p=128)
        wklr = wkl.rearrange("(o p) n -> p o n", p=128)
        wvhr = wvh.rearrange("(o p) n -> p o n", p=128)
        wvlr = wvl.rearrange("(o p) n -> p o n", p=128)
        xhr = xh.rearrange("(o p) s -> p o s", p=128)
        xlr = xl.rearrange("(o p) s -> p o s", p=128)

        def rope(pcs, b_t, rr_t, js, act_ok=True):  # generator: yields mid-way
            """evens' = (e+b0)*cos - (o+b1)*sin ; odds' = (e+b0)*sin + (o+b1)*cos
            writes bf16 into rr_t ([all evens | all odds] chunks)."""
            cp0 = tmp.tile([128, 512], BF16, name="cp0", tag="tt")
            if act_ok:
                nc.scalar.activation(cp0[:], pcs[0][:], AF.Identity,
                                     bias=b_t[:, 0:1])
            else:
                nc.vector.tensor_scalar_add(cp0[:], pcs[0][:], b_t[:, 0:1])
            t1 = tmp.tile([128, 512], BF16, name="t1", tag="tt")
            nc.vector.tensor_mul(t1[:], cp0[:], cos_t[:, js])
            t3 = tmp.tile([128, 512], BF16, name="t3", tag="tt")
            nc.vector.tensor_mul(t3[:], cp0[:], sin_t[:, js])
            yield
            cp1 = tmp.tile([128, 512], BF16, name="cp1", tag="tt")
            nc.vector.tensor_scalar_add(cp1[:], pcs[1][:], b_t[:, 1:2])
            t2 = tmp.tile([128, 512], BF16, name="t2", tag="tt")
            nc.vector.tensor_mul(t2[:], cp1[:], sin_t[:, js])
            nc.vector.tensor_sub(rr_t[:, 0, js], t1[:], t2[:])
            yield
            t4 = tmp.tile([128, 512], BF16, name="t4", tag="tt")
            nc.vector.tensor_mul(t4[:], cp1[:], cos_t[:, js])
            nc.vector.tensor_add(rr_t[:, 1, js], t3[:], t4[:])
            yield

        def regroup(rr_t, hh_t, js, eng=None):
            """[all evens | all odds] -> head-contiguous, per 32-row block.
            dst chunk cc rows: [h2cc e, h2cc o, h2cc+1 e, h2cc+1 o]."""
            eng = eng or nc.vector
            for cc in range(2):
                for a in range(2):
                    src = slice(64 * cc + 32 * a, 64 * cc + 32 * a + 32)
                    for eo in range(2):
                        dst = slice(64 * a + 32 * eo, 64 * a + 32 * eo + 32)
                        eng.tensor_copy(hh_t[dst, cc, js],
                                        rr_t[src, eo, js])
                yield

        outr = out.rearrange("(sb p) n -> sb p n", p=128)
        consts_loaded = []
        xq_tiles = {}

        def get_xq(qi):
            """Allocate + start the x DMAs for quarter qi (idempotent)."""
            if qi in xq_tiles:
                return xq_tiles[qi]
            js = bass.ts(qi, 512)
            xqh = xp.tile([128, KS, 512], FP8H, name="xqh", tag="xh")
            xql = xp.tile([128, KS, 512], FP8L, name="xql", tag="xl")
            if qi == 0:
                # interleave x/wq pieces so the first DR matmuls start on
                # the first two k-tiles as early as possible; xl and wv go
                # on the scalar ring in parallel
                nc.sync.dma_start(xqh[:, 0:2], xhr[:, 0:2, js])
                nc.sync.dma_start(wqh_t[:, 2:8], wqhr[:, 2:8])
                nc.sync.dma_start(xqh[:, 2:8], xhr[:, 2:8, js])
                nc.sync.dma_start(wql_t[:], wqlr[:])
                nc.scalar.dma_start(xql[:], xlr[:, :, js])
                nc.scalar.dma_start(wvh_t[:], wvhr[:])
                nc.scalar.dma_start(wvl_t[:], wvlr[:])
            else:
                nc.sync.dma_start(xqh[:], xhr[:, :, js])
                nc.sync.dma_start(xql[:], xlr[:, :, js])
            xq_tiles[qi] = (xqh, xql)
            return xq_tiles[qi]

        # early loads, in true dependency order (SP HWDGE ring is FIFO).
        nc.sync.dma_start(wqh_t[:, 0:2], wqhr[:, 0:2])
        # (xq quarter 0 DMAs are emitted next, from gen_proj(0) below)

        # warm-up: keep the PE busy while the first DMAs land so the
        # p-state ramp finishes before real work arrives.  The operand
        # region is zeroed first (uninitialized SBUF can hold NaN bit
        # patterns which would otherwise leak through psum slot reuse).
        nc.gpsimd.memset(qr_t[:, 0, 0:128], 0.0)
        wu = ps.tile([128, 512], F32, name="wu", tag="ps")
        for r in range(36):
            nc.tensor.matmul(wu[:, 0:128], qr_t[:, 0, 0:128],
                             qr_t[:, 0, 0:128],
                             start=(r == 0), stop=(r == 35))

        def dr_chain(p_out, groups, c_slice, w_stationary, yield_at=(4, 8)):
            """12 DoubleRow matmuls accumulating the 3-term hi/lo expansion."""
            n = 0
            for (wt, xt) in groups:
                for t in range(4):
                    if w_stationary:
                        lhsT = wt[:, 2 * t:2 * t + 2, c_slice]
                        rhs = xt[:, 2 * t:2 * t + 2, :]
                        o = p_out
                    else:
                        lhsT = xt[:, 2 * t:2 * t + 2, c_slice]
                        rhs = wt[:, 2 * t:2 * t + 2, :]
                        o = p_out
                    nc.tensor.matmul(o, lhsT, rhs,
                                     start=(n == 0), stop=(n == 11),
                                     perf_mode=DR)
                    n += 1
                    if n in yield_at:
                        yield

        def gen_proj(qi):
            """Projections + RoPE + head-regroup for quarter qi."""
            js = bass.ts(qi, 512)
            xqh, xql = get_xq(qi)
            if qi == 0:
                nc.sync.dma_start(bq_t[:], bqp)
                nc.gpsimd.dma_start(cos_t[:], cos4)
                nc.gpsimd.dma_start(sin_t[:], sin4)
                nc.gpsimd.dma_start(wkh_t[:], wkhr[:])
                nc.gpsimd.dma_start(wkl_t[:], wklr[:])
                nc.sync.dma_start(bk_t[:], bkp)
                nc.gpsimd.memset(v1_t[:, :, :, D], 1.0)
            qcs = []
            for c in range(2):
                p = ps.tile([128, 512], F32, name="psA", tag="ps")
                qgrp = [(wqh_t, xqh), (wqh_t, xql), (wql_t, xqh)]
                for _ in dr_chain(p[:], qgrp, bass.ts(c, 128), True):
                    yield
                qcs.append(p)
                yield
            for _ in rope(qcs, bq_t, qr_t, js, act_ok=(qi < 2)):
                yield
            kcs = []
            for c in range(2):
                p = ps.tile([128, 512], F32, name="psA", tag="ps")
                kgrp = [(wkh_t, xqh), (wkh_t, xql), (wkl_t, xqh)]
                for _ in dr_chain(p[:], kgrp, bass.ts(c, 128), True):
                    yield
                kcs.append(p)
                yield
            for _ in rope(kcs, bk_t, kr_t, js, act_ok=(qi < 2)):
                yield
            for _ in regroup(qr_t, qh_t, js, eng=nc.gpsimd):
                yield
            for _ in regroup(kr_t, kh_t, js, eng=nc.gpsimd):
                yield
            for sl in range(4):
                sb = 4 * qi + sl
                p = ps.tile([128, 512], F32, name="psAv", tag="ps")
                vgrp = [(wvh_t, xqh), (wvh_t, xql), (wvl_t, xqh)]
                for _ in dr_chain(p[:, :DG], vgrp, bass.ts(sl, 128), False):
                    yield
                vsl = p[:, :DG].rearrange("p (h d) -> p h d", d=D)
                if qi <= 1:
                    nc.scalar.activation(v1_t[:, sb, :, 0:D], vsl, AF.Copy)
                else:
                    nc.vector.tensor_copy(v1_t[:, sb, :, 0:D], vsl)
                yield
            # prefetch next quarter's x while this quarter's attention runs
            if qi + 1 < NQ:
                get_xq(qi + 1)

        def gen_scores(j):
            """Scores + exp for sq-quarter j, per head pair cc."""
            if not consts_loaded:
                consts_loaded.append(1)
                nc.gpsimd.dma_start(tri_t[:], trid)
                nc.gpsimd.dma_start(id_t[:], idd)
                for k in range(2):
                    nc.gpsimd.dma_start(
                        wo_t[:, k],
                        wo.rearrange("(o p) n -> p o n", p=128)[:, k])
            nblk = 4 * j + 4
            for cc in range(2):
                et = et_tiles[(2 * j + cc) % 3]
                for i in range(nblk):
                    c0 = max(0, 128 * i - 512 * j)
                    spb = psc.tile([128, 2, 512], F32, name="sp", tag="sc")
                    for a in range(2):
                        hp = slice(64 * a, 64 * a + 64)
                        nc.tensor.matmul(spb[:, a, c0:512],
                                         kh_t[hp, cc, bass.ts(i, 128)],
                                         qh_t[hp, cc,
                                              512 * j + c0:512 * (j + 1)],
                                         start=True, stop=True)
                    nc.scalar.activation(et[:, i, :, c0:512],
                                         spb[:, :, c0:512], AF.Exp,
                                         scale=0.125)
                    if 128 * i - 512 * j >= 0:
                        # diagonal block: zero the masked lower triangle
                        nc.gpsimd.tensor_mul(et[:, i, :, c0:c0 + 128],
                                             et[:, i, :, c0:c0 + 128],
                                             tri_t[:])
                    yield

        def gen_pv(j, ccs=(0, 1), chase=None):
            """PV + normalize + transpose for quarter j, head pairs `ccs`.
            `chase` units (out-proj of this quarter) are emitted right after
            each cc=1 transpose, when both k-chunks of that s-block exist."""
            for cc in ccs:
                et = et_tiles[(2 * j + cc) % 3]
                for qt in range(4):
                    nb = 4 * j + qt + 1
                    qs = 128 * qt
                    # the very last subtile normalizes half a as soon as its
                    # chain lands, shortening the drain-tail critical path
                    eager = (j == 3 and cc == 1 and qt == 3)
                    pv = ppv.tile([128, 512], F32, name="pv", tag="pv")
                    rc = nrm.tile([128, 2], F32, name="rc")
                    o_sb = stg.tile([128, 128], BF16, name="osb", tag="osb")
                    for a in range(2):
                        for i in range(nb):
                            nc.tensor.matmul(
                                pv[:, 256 * a:256 * a + 65],
                                et[:, i, a, qs:qs + 128],
                                v1_t[:, i, 2 * cc + a, :],
                                start=(a == 0 and i == 0),
                                stop=(a == 1 and i == nb - 1))
                            if i % 8 == 7:
                                yield
                        if eager:
                            nc.vector.reciprocal(
                                rc[:, a:a + 1], pv[:, 256 * a + 64:
                                                   256 * a + 65])
                            nc.vector.tensor_scalar_mul(
                                o_sb[:, 64 * a:64 * a + 64],
                                pv[:, 256 * a:256 * a + 64], rc[:, a:a + 1])
                        yield
                    if not eager:
                        nc.vector.reciprocal(
                            rc[:],
                            pv.rearrange("p (t c) -> p t c", t=2)[:, :, 64])
                        for a in range(2):
                            nc.vector.tensor_scalar_mul(
                                o_sb[:, 64 * a:64 * a + 64],
                                pv[:, 256 * a:256 * a + 64], rc[:, a:a + 1])
                        yield
                    pt = ppv.tile([128, 512], F32, name="pt", tag="pv")
                    ptv = pt[:, 0:64].bitcast(BF16)
                    nc.tensor.transpose(ptv, o_sb[:], id_t[:])
                    nc.vector.tensor_copy(
                        o_t[:, cc, 512 * j + qs:512 * j + qs + 128], ptv)
                    yield
                    if chase is not None and cc == 1:
                        for _ in chase:
                            break

        def gen_outproj(j, late=False):
            # late: the proj (ps) and scores (psc) pools are both idle --
            # cycle them so two psum pairs stay in flight.  st copies stay
            # on DVE while the exp stream still occupies ACT (an ACT copy
            # would queue behind the remaining exps, holding its psum slot);
            # only the final piece uses ACT, after the last exp has drained.
            # Non-late units yield between the two halves so a single ps
            # slot is held at a time (the proj chains share that pool).
            cyc = [(ps, "ps"), (psc, "sc")]
            for sl in range(4):
                sb = 4 * j + sl
                if late:
                    p0p, t0 = cyc[sl % 2]
                    p1p, t1 = cyc[(sl + 1) % 2]
                else:
                    (p0p, t0), (p1p, t1) = cyc[0], cyc[0]
                ps0 = p0p.tile([128, 512], F32, name="psC0", tag=t0)
                ps1 = p1p.tile([128, 512], F32, name="psC1", tag=t1)
                for k in range(2):
                    nc.tensor.matmul(ps0[:], o_t[:, k, bass.ts(sb, 128)],
                                     wo_t[:, k, 0:512],
                                     start=(k == 0), stop=(k == 1))
                for k in range(2):
                    nc.tensor.matmul(ps1[:], o_t[:, k, bass.ts(sb, 128)],
                                     wo_t[:, k, 512:1024],
                                     start=(k == 0), stop=(k == 1))
                st = stg.tile([128, 1024], BF16, name="st", tag="st")
                if late:
                    nc.scalar.activation(st[:, 0:512], ps0[:], AF.Copy)
                else:
                    nc.vector.tensor_copy(st[:, 0:512], ps0[:])
                nc.gpsimd.dma_start(outr[sb][:, 0:512], st[:, 0:512])
                if late and sl == 3:
                    # final piece: 256-col halves on two rings so the last
                    # copy+transfer in the drain tail is half-length
                    nc.scalar.activation(st[:, 512:768], ps1[:, 0:256],
                                         AF.Copy)
                    nc.sync.dma_start(outr[sb][:, 512:768], st[:, 512:768])
                    nc.vector.tensor_copy(st[:, 768:1024], ps1[:, 256:512])
                    nc.gpsimd.dma_start(outr[sb][:, 768:1024],
                                        st[:, 768:1024])
                else:
                    nc.vector.tensor_copy(st[:, 512:1024], ps1[:])
                    nc.sync.dma_start(outr[sb][:, 512:1024], st[:, 512:1024])
                yield

        def drain(g):
            for _ in g:
                pass

        def weave(primary, *others, stride=1):
            """Emit `primary` units round-robin with pieces from `others`."""
            gens = [g for g in others if g is not None]
            sts = list(stride) if isinstance(stride, (tuple, list)) \
                else [stride] * len(gens)
            n = 0
            while True:
                try:
                    next(primary)
                    n += 1
                except StopIteration:
                    break
                for g in list(gens):
                    st = sts[gens.index(g)]
                    reps = int(round(1 / st)) if st < 1 else \
                        (1 if n % st == 0 else 0)
                    for _ in range(reps):
                        try:
                            next(g)
                        except StopIteration:
                            idx = gens.index(g)
                            gens.pop(idx)
                            sts.pop(idx)
                            break
            for g in gens:
                for _ in g:
                    pass

        def chain(*gens):
            for g in gens:
                yield from g

        # software-pipelined emission.  PE executes in order, so nothing
        # that stalls on a late exp may precede ready work: pv(j) cc=0
        # follows scores(j) (its exps are done by then), while pv(j) cc=1
        # is deferred past scores(j+1) (its exps complete while scores(j+1)
        # runs).  outproj(0..2) fills quarter 3's exp-saturated window;
        # outproj(3) chases the cc=1 transposes inside pv(3).
        drain(gen_proj(0))
        weave(chain(gen_scores(0), gen_pv(0, ccs=(0,))), gen_proj(1))
        weave(chain(gen_scores(1), gen_pv(1, ccs=(0,))),
              chain(gen_pv(0, ccs=(1,)), gen_proj(2)), stride=2)
        weave(chain(gen_scores(2), gen_pv(2, ccs=(0,))),
              chain(gen_pv(1, ccs=(1,)), gen_proj(3), gen_outproj(0)),
              stride=1)
        weave(chain(gen_scores(3), gen_pv(3, ccs=(0,))),
              chain(gen_pv(2, ccs=(1,)), gen_outproj(1)), stride=3)
        drain(gen_outproj(2))
        drain(gen_pv(3, ccs=(1,), chase=gen_outproj(3, late=True)))

    nc.compile()
    return nc


_EO_IDX = None


def _eo_index():
    """Column permutation within one head group: all even components of the
    4 heads first (h-major), then all odd components."""
    global _EO_IDX
    if _EO_IDX is None:
        idx = []
        for eo in (0, 1):
            for h in range(HPG):
                idx.extend(range(64 * h + eo, 64 * h + 64, 2))
        _EO_IDX = np.asarray(idx)
    return _EO_IDX


def _split8(a):
    import ml_dtypes
    hi = np.asarray(a, dtype=np.float32).astype(ml_dtypes.float8_e4m3)
    lo = (np.asarray(a, dtype=np.float32) - hi.astype(np.float32)).astype(
        ml_dtypes.float8_e5m2)
    return np.ascontiguousarray(hi), np.ascontiguousarray(lo)


def make_in_maps(x, Wq, bq, Wk, bk, Wv, bv, Wo, bo, mask, freqs_cos, freqs_sin):
    import ml_dtypes
    idx = _eo_index()
    f32 = np.float32
    bf16 = ml_dtypes.bfloat16
    cosT = np.ascontiguousarray(freqs_cos.T, dtype=f32)       # (32, S)
    sinT = np.ascontiguousarray(freqs_sin.T, dtype=f32)
    cos4 = np.tile(cosT, (4, 1)).astype(bf16)                 # (128, S)
    sin4 = np.tile(sinT, (4, 1)).astype(bf16)
    # upper-triangular (incl. diagonal) ones: tri[k, q] = 1 iff k <= q
    tri = np.broadcast_to(np.triu(np.ones((128, 128), f32))[:, None, :],
                          (128, 2, 128)).copy().astype(bf16)
    idm = np.eye(128, dtype=f32).astype(bf16)

    in_maps = []
    for core in range(NCORES):
        b, g = core // G, core % G
        cols = slice(DG * g, DG * (g + 1))
        wq_g = np.ascontiguousarray(Wq[:, cols][:, idx], dtype=f32)
        wk_g = np.ascontiguousarray(Wk[:, cols][:, idx], dtype=f32)
        wv_g = np.ascontiguousarray(Wv[:, cols], dtype=f32)
        wqh_g, wql_g = _split8(wq_g)
        wkh_g, wkl_g = _split8(wk_g)
        wvh_g, wvl_g = _split8(wv_g)
        wo_g = np.ascontiguousarray(Wo[cols, :], dtype=f32).astype(bf16)
        bq_g = np.ascontiguousarray(
            bq[cols][idx].reshape(2, 128).T, dtype=f32)
        bk_g = np.ascontiguousarray(
            bk[cols][idx].reshape(2, 128).T, dtype=f32)
        xT_b = np.ascontiguousarray(np.asarray(x[b], dtype=f32).T)
        xh_b, xl_b = _split8(xT_b)
        in_maps.append(dict(xh=xh_b, xl=xl_b,
                            wqh=wqh_g, wql=wql_g, wkh=wkh_g, wkl=wkl_g,
                            wvh=wvh_g, wvl=wvl_g, wo=wo_g,
                            bqp=bq_g, bkp=bk_g, cos4=cos4, sin4=sin4,
                            trid=tri, idd=idm))
    return in_maps


_NC_CACHE = None
LAST_RESULTS = None


def kernel(**inputs):
    global _NC_CACHE
    if _NC_CACHE is None:
        _NC_CACHE = build_program()
    nc = _NC_CACHE

    inputs = {k: np.asarray(v) for k, v in inputs.items()}
    in_maps = make_in_maps(**inputs)
    kwargs = {}
    if os.environ.get("BASS_TRACE"):
        kwargs = dict(trace=True, trace_cores=list(range(NCORES)),
                      stitch_traces=True)
    res = run_bass_kernel_spmd(nc, in_maps, core_ids=list(range(NCORES)),
                               **kwargs)
    global LAST_RESULTS
    LAST_RESULTS = res

    out = np.zeros((B, S, HID), np.float32)
    for core in range(NCORES):
        out[core // G] += res.results[core]["out"].astype(np.float32)
    out += inputs["bo"].astype(np.float32)
    out += (inputs["bv"].astype(np.float32) @ inputs["Wo"].astype(np.float32))
    return out


# revision 38
# speedup vs baseline: 1.0126x; 1.0126x over previous
"""Multi-head causal attention (B=2, S=2048, H=16, D=64) on 8 TRN2 NeuronCores.

Sharding: data-parallel over batch (2) x tensor-parallel over head groups (4).
Core c handles batch b = c // 4, head group g = c % 4 (heads 4g..4g+3).
Each core computes q/k/v projections for its 4 heads, RoPE, causal
flash-style attention (upper-triangular blocks skipped), and a partial
output projection out_partial = attn_out @ Wo[256g:256g+256].  The host
sums the 4 partials per batch and adds the (bias) terms.

Key layout/precision choices:
 - q/k/v projections run in fp8 DoubleRow mode (0.5 PE cycles/row, two
   128-row k-tiles per instruction).  Accuracy is preserved with a
   hi/lo split: x = x_hi(e4m3) + x_lo(e5m2), W = W_hi + W_lo, and the
   product is the 3-term expansion x_hi*W_hi + x_hi*W_lo + x_lo*W_hi
   accumulated in one PSUM group (12 DR matmuls per 1024-deep chain,
   vs 8 full-rate bf16 matmuls) -- 25% fewer PE cycles than bf16 at
   slightly BETTER precision (lo term recovers the e4m3 rounding).
 - q/k are computed TRANSPOSED (d on partitions); Wq/Wk columns are
   permuted to [all even comps | all odd comps] so RoPE runs as
   full-128-partition DVE ops; a regroup pass makes them head-contiguous.
 - scores are computed transposed (kv on partitions, q free) in bf16.
 - exp(scores) goes to per-quarter persistent SBUF tiles (one per head
   pair).  Causal masking is a post-exp multiply of the diagonal blocks
   by a 0/1 triangle on gpsimd.
 - PV uses exp(scores) as the STATIONARY operand ([kv, q-subtile]) and
   v (plus an appended ones column) as the 65-wide MOVING operand:
   the PE charge is 65 cycles per accumulation step instead of up to
   512 -- half the PV cost of the scores-as-moving mapping.  The ones
   column makes psum col 64 the softmax denominator, per q-partition,
   so normalization is one tiny reciprocal + per-partition
   tensor_scalar multiplies.
 - the normalized [q, hd] tile is transposed back to [hd, q] with a
   128x128 PE transpose against an identity (128 cycles) and copied to
   the bf16 attn-out tile consumed by the out-projection.
 - out-projections (bf16 o x bf16 Wo) are deferred into quarter 3 and
   the epilogue, where ACT is exp-saturated / idle and the proj psum
   pool is free (PSUM budget: proj 2 + scores 4 + pv/pt 2 = 8 banks).
"""

import os
import numpy as np
from contextlib import ExitStack

import concourse.bass as bass
import concourse.tile as tile
from concourse import bacc, mybir
from concourse.bass_utils import run_bass_kernel_spmd

F32 = mybir.dt.float32
BF16 = mybir.dt.bfloat16
FP8H = mybir.dt.float8e4
FP8L = mybir.dt.float8e5
AF = mybir.ActivationFunctionType
DR = mybir.MatmulPerfMode.DoubleRow

B, S, H, D = 2, 2048, 16, 64
HID = H * D           # 1024
NCORES = 8
G = 4                 # head groups
HPG = H // G          # heads per group = 4
DG = HPG * D          # per-group model dim = 256
KS = HID // 128       # 8 k-subtiles
NQ = 4                # S quarters (chunks of 512)
SB = S // 128         # 16 s-blocks


def build_program():
    nc = bacc.Bacc("TRN2", target_bir_lowering=False, debug=False,
                   num_devices=NCORES)

    xh = nc.dram_tensor("xh", [HID, S], FP8H, kind="ExternalInput").ap()
    xl = nc.dram_tensor("xl", [HID, S], FP8L, kind="ExternalInput").ap()
    wqh = nc.dram_tensor("wqh", [HID, DG], FP8H, kind="ExternalInput").ap()
    wql = nc.dram_tensor("wql", [HID, DG], FP8L, kind="ExternalInput").ap()
    wkh = nc.dram_tensor("wkh", [HID, DG], FP8H, kind="ExternalInput").ap()
    wkl = nc.dram_tensor("wkl", [HID, DG], FP8L, kind="ExternalInput").ap()
    wvh = nc.dram_tensor("wvh", [HID, DG], FP8H, kind="ExternalInput").ap()
    wvl = nc.dram_tensor("wvl", [HID, DG], FP8L, kind="ExternalInput").ap()
    wo = nc.dram_tensor("wo", [DG, HID], BF16, kind="ExternalInput").ap()
    bqp = nc.dram_tensor("bqp", [128, 2], F32, kind="ExternalInput").ap()
    bkp = nc.dram_tensor("bkp", [128, 2], F32, kind="ExternalInput").ap()
    cos4 = nc.dram_tensor("cos4", [128, S], BF16, kind="ExternalInput").ap()
    sin4 = nc.dram_tensor("sin4", [128, S], BF16, kind="ExternalInput").ap()
    trid = nc.dram_tensor("trid", [128, 2, 128], BF16, kind="ExternalInput").ap()
    idd = nc.dram_tensor("idd", [128, 128], BF16, kind="ExternalInput").ap()
    out = nc.dram_tensor("out", [S, HID], BF16, kind="ExternalOutput").ap()

    with tile.TileContext(nc) as tc, ExitStack() as ctx:
        const = ctx.enter_context(tc.tile_pool(name="const", bufs=1))
        xp = ctx.enter_context(tc.tile_pool(name="xp", bufs=2))
        tmp = ctx.enter_context(tc.tile_pool(name="tmp", bufs=6))
        stg = ctx.enter_context(tc.tile_pool(name="stg", bufs=4))
        nrm = ctx.enter_context(tc.tile_pool(name="nrm", bufs=2))
        ps = ctx.enter_context(tc.tile_pool(name="ps", bufs=2, space="PSUM"))
        psc = ctx.enter_context(tc.tile_pool(name="psc", bufs=2, space="PSUM"))
        ppv = ctx.enter_context(tc.tile_pool(name="ppv", bufs=2, space="PSUM"))

        # ---- persistent SBUF tiles (DMAs emitted at first-use points) ----
        wqh_t = const.tile([128, KS, DG], FP8H)
        wql_t = const.tile([128, KS, DG], FP8L)
        wkh_t = const.tile([128, KS, DG], FP8H)
        wkl_t = const.tile([128, KS, DG], FP8L)
        wvh_t = const.tile([128, KS, DG], FP8H)
        wvl_t = const.tile([128, KS, DG], FP8L)
        wo_t = const.tile([128, 2, HID], BF16)
        cos_t = const.tile([128, S], BF16)
        sin_t = const.tile([128, S], BF16)
        tri_t = const.tile([128, 2, 128], BF16)
        id_t = const.tile([128, 128], BF16)
        bq_t = const.tile([128, 2], F32)
        bk_t = const.tile([128, 2], F32)
        v1_t = const.tile([128, SB, HPG, D + 1], BF16)   # v blocks + ones col
        qr_t = const.tile([128, 2, S], BF16)   # roped q, [evens|odds] chunks
        kr_t = const.tile([128, 2, S], BF16)
        qh_t = const.tile([128, 2, S], BF16)   # head-contiguous roped q
        kh_t = const.tile([128, 2, S], BF16)
        o_t = const.tile([128, 2, S], BF16)    # attn outT (hd on partitions)
        # exp(scores) tiles [kv-block, head-in-pair, q-col]; quarter j head
        # pair cc uses tile (2j+cc) % 3 -- the 3-way rotation lets pv of the
        # second head pair defer into the NEXT quarter's window without a
        # write-after-read race against that quarter's exps.
        et0_t = const.tile([128, SB, 2, 512], BF16)
        et1_t = const.tile([128, SB, 2, 512], BF16)
        et2_t = const.tile([128, SB, 2, 512], BF16)
        et_tiles = [et0_t, et1_t, et2_t]

        wqhr = wqh.rearrange("(o p) n -> p o n", p=128)
        wqlr = wql.rearrange("(o p) n -> p o n", p=128)
        wkhr = wkh.rearrange("(o p) n -> p o n", p=128)
        wklr = wkl.rearrange("(o p) n -> p o n", p=128)
        wvhr = wvh.rearrange("(o p) n -> p o n", p=128)
        wvlr = wvl.rearrange("(o p) n -> p o n", p=128)
        xhr = xh.rearrange("(o p) s -> p o s", p=128)
        xlr = xl.rearrange("(o p) s -> p o s", p=128)

        def rope(pcs, b_t, rr_t, js, act_ok=True):  # generator: yields mid-way
            """evens' = (e+b0)*cos - (o+b1)*sin ; odds' = (e+b0)*sin + (o+b1)*cos
            writes bf16 into rr_t ([all evens | all odds] chunks)."""
            cp0 = tmp.tile([128, 512], BF16, name="cp0", tag="tt")
            if act_ok:
                nc.scalar.activation(cp0[:], pcs[0][:], AF.Identity,
                                     bias=b_t[:, 0:1])
            else:
                nc.vector.tensor_scalar_add(cp0[:], pcs[0][:], b_t[:, 0:1])
            t1 = tmp.tile([128, 512], BF16, name="t1", tag="tt")
            nc.vector.tensor_mul(t1[:], cp0[:], cos_t[:, js])
            t3 = tmp.tile([128, 512], BF16, name="t3", tag="tt")
            nc.vector.tensor_mul(t3[:], cp0[:], sin_t[:, js])
            yield
            cp1 = tmp.tile([128, 512], BF16, name="cp1", tag="tt")
            nc.vector.tensor_scalar_add(cp1[:], pcs[1][:], b_t[:, 1:2])
            t2 = tmp.tile([128, 512], BF16, name="t2", tag="tt")
            nc.vector.tensor_mul(t2[:], cp1[:], sin_t[:, js])
            nc.vector.tensor_sub(rr_t[:, 0, js], t1[:], t2[:])
            yield
            t4 = tmp.tile([128, 512], BF16, name="t4", tag="tt")
            nc.vector.tensor_mul(t4[:], cp1[:], cos_t[:, js])
            nc.vector.tensor_add(rr_t[:, 1, js], t3[:], t4[:])
            yield

        def regroup(rr_t, hh_t, js, eng=None):
            """[all evens | all odds] -> head-contiguous, per 32-row block.
            dst chunk cc rows: [h2cc e, h2cc o, h2cc+1 e, h2cc+1 o]."""
            eng = eng or nc.vector
            for cc in range(2):
                for a in range(2):
                    src = slice(64 * cc + 32 * a, 64 * cc + 32 * a + 32)
                    for eo in range(2):
                        dst = slice(64 * a + 32 * eo, 64 * a + 32 * eo + 32)
                        eng.tensor_copy(hh_t[dst, cc, js],
                                        rr_t[src, eo, js])
                yield

        outr = out.rearrange("(sb p) n -> sb p n", p=128)
        consts_loaded = []
        xq_tiles = {}

        def get_xq(qi):
            """Allocate + start the x DMAs for quarter qi (idempotent)."""
            if qi in xq_tiles:
                return xq_tiles[qi]
            js = bass.ts(qi, 512)
            xqh = xp.tile([128, KS, 512], FP8H, name="xqh", tag="xh")
            xql = xp.tile([128, KS, 512], FP8L, name="xql", tag="xl")
            if qi == 0:
                # interleave x/wq pieces so the first DR matmuls start on
                # the first two k-tiles as early as possible; xl and wv go
                # on the scalar ring in parallel
                nc.sync.dma_start(xqh[:, 0:2], xhr[:, 0:2, js])
                nc.sync.dma_start(wqh_t[:, 2:8], wqhr[:, 2:8])
                nc.sync.dma_start(xqh[:, 2:8], xhr[:, 2:8, js])
                nc.sync.dma_start(wql_t[:], wqlr[:])
                nc.scalar.dma_start(xql[:], xlr[:, :, js])
                nc.scalar.dma_start(wvh_t[:], wvhr[:])
                nc.scalar.dma_start(wvl_t[:], wvlr[:])
            else:
                nc.sync.dma_start(xqh[:], xhr[:, :, js])
                nc.sync.dma_start(xql[:], xlr[:, :, js])
            xq_tiles[qi] = (xqh, xql)
            return xq_tiles[qi]

        # early loads, in true dependency order (SP HWDGE ring is FIFO).
        nc.sync.dma_start(wqh_t[:, 0:2], wqhr[:, 0:2])
        # (xq quarter 0 DMAs are emitted next, from gen_proj(0) below)

        # warm-up: keep the PE busy while the first DMAs land so the
        # p-state ramp finishes before real work arrives.  The operand
        # region is zeroed first (uninitialized SBUF can hold NaN bit
        # patterns which would otherwise leak through psum slot reuse).
        nc.gpsimd.memset(qr_t[:, 0, 0:128], 0.0)
        wu = ps.tile([128, 512], F32, name="wu", tag="ps")
        for r in range(36):
            nc.tensor.matmul(wu[:, 0:128], qr_t[:, 0, 0:128],
                             qr_t[:, 0, 0:128],
                             start=(r == 0), stop=(r == 35))

        def dr_chain(p_out, groups, c_slice, w_stationary, yield_at=(4, 8)):
            """12 DoubleRow matmuls accumulating the 3-term hi/lo expansion."""
            n = 0
            for (wt, xt) in groups:
                for t in range(4):
                    if w_stationary:
                        lhsT = wt[:, 2 * t:2 * t + 2, c_slice]
                        rhs = xt[:, 2 * t:2 * t + 2, :]
                        o = p_out
                    else:
                        lhsT = xt[:, 2 * t:2 * t + 2, c_slice]
                        rhs = wt[:, 2 * t:2 * t + 2, :]
                        o = p_out
                    nc.tensor.matmul(o, lhsT, rhs,
                                     start=(n == 0), stop=(n == 11),
                                     perf_mode=DR)
                    n += 1
                    if n in yield_at:
                        yield

        def gen_proj(qi):
            """Projections + RoPE + head-regroup for quarter qi."""
            js = bass.ts(qi, 512)
            xqh, xql = get_xq(qi)
            if qi == 0:
                nc.sync.dma_start(bq_t[:], bqp)
                nc.gpsimd.dma_start(cos_t[:], cos4)
                nc.gpsimd.dma_start(sin_t[:], sin4)
                nc.gpsimd.dma_start(wkh_t[:], wkhr[:])
                nc.gpsimd.dma_start(wkl_t[:], wklr[:])
                nc.sync.dma_start(bk_t[:], bkp)
                nc.gpsimd.memset(v1_t[:, :, :, D], 1.0)
            qcs = []
            for c in range(2):
                p = ps.tile([128, 512], F32, name="psA", tag="ps")
                qgrp = [(wqh_t, xqh), (wqh_t, xql), (wql_t, xqh)]
                for _ in dr_chain(p[:], qgrp, bass.ts(c, 128), True):
                    yield
                qcs.append(p)
                yield
            for _ in rope(qcs, bq_t, qr_t, js, act_ok=(qi < 2)):
                yield
            kcs = []
            for c in range(2):
                p = ps.tile([128, 512], F32, name="psA", tag="ps")
                kgrp = [(wkh_t, xqh), (wkh_t, xql), (wkl_t, xqh)]
                for _ in dr_chain(p[:], kgrp, bass.ts(c, 128), True):
                    yield
                kcs.append(p)
                yield
            for _ in rope(kcs, bk_t, kr_t, js, act_ok=(qi < 2)):
                yield
            for _ in regroup(qr_t, qh_t, js, eng=nc.gpsimd):
                yield
            for _ in regroup(kr_t, kh_t, js, eng=nc.gpsimd):
                yield
            for sl in range(4):
                sb = 4 * qi + sl
                p = ps.tile([128, 512], F32, name="psAv", tag="ps")
                vgrp = [(wvh_t, xqh), (wvh_t, xql), (wvl_t, xqh)]
                for _ in dr_chain(p[:, :DG], vgrp, bass.ts(sl, 128), False):
                    yield
                vsl = p[:, :DG].rearrange("p (h d) -> p h d", d=D)
                if qi <= 1:
                    nc.scalar.activation(v1_t[:, sb, :, 0:D], vsl, AF.Copy)
                else:
                    nc.vector.tensor_copy(v1_t[:, sb, :, 0:D], vsl)
                yield
            # prefetch next quarter's x while this quarter's attention runs
            if qi + 1 < NQ:
                get_xq(qi + 1)

        def gen_scores(j):
            """Scores + exp for sq-quarter j, per head pair cc."""
            if not consts_loaded:
                consts_loaded.append(1)
                nc.gpsimd.dma_start(tri_t[:], trid)
                nc.gpsimd.dma_start(id_t[:], idd)
                for k in range(2):
                    nc.gpsimd.dma_start(
                        wo_t[:, k],
                        wo.rearrange("(o p) n -> p o n", p=128)[:, k])
            nblk = 4 * j + 4
            for cc in range(2):
                et = et_tiles[(2 * j + cc) % 3]
                for i in range(nblk):
                    c0 = max(0, 128 * i - 512 * j)
                    spb = psc.tile([128, 2, 512], F32, name="sp", tag="sc")
                    for a in range(2):
                        hp = slice(64 * a, 64 * a + 64)
                        nc.tensor.matmul(spb[:, a, c0:512],
                                         kh_t[hp, cc, bass.ts(i, 128)],
                                         qh_t[hp, cc,
                                              512 * j + c0:512 * (j + 1)],
                                         start=True, stop=True)
                    nc.scalar.activation(et[:, i, :, c0:512],
                                         spb[:, :, c0:512], AF.Exp,
                                         scale=0.125)
                    if 128 * i - 512 * j >= 0:
                        # diagonal block: zero the masked lower triangle
                        nc.gpsimd.tensor_mul(et[:, i, :, c0:c0 + 128],
                                             et[:, i, :, c0:c0 + 128],
                                             tri_t[:])
                    yield

        def gen_pv(j, ccs=(0, 1), chase=None):
            """PV + normalize + transpose for quarter j, head pairs `ccs`.
            `chase` units (out-proj of this quarter) are emitted right after
            each cc=1 transpose, when both k-chunks of that s-block exist."""
            for cc in ccs:
                et = et_tiles[(2 * j + cc) % 3]
                for qt in range(4):
                    nb = 4 * j + qt + 1
                    qs = 128 * qt
                    pv = ppv.tile([128, 512], F32, name="pv", tag="pv")
                    for a in range(2):
                        for i in range(nb):
                            nc.tensor.matmul(
                                pv[:, 256 * a:256 * a + 65],
                                et[:, i, a, qs:qs + 128],
                                v1_t[:, i, 2 * cc + a, :],
                                start=(a == 0 and i == 0),
                                stop=(a == 1 and i == nb - 1))
                            if i % 8 == 7:
                                yield
                        yield
                    rc = nrm.tile([128, 2], F32, name="rc")
                    nc.vector.reciprocal(
                        rc[:], pv.rearrange("p (t c) -> p t c", t=2)[:, :, 64])
                    o_sb = stg.tile([128, 128], BF16, name="osb", tag="osb")
                    for a in range(2):
                        nc.vector.tensor_scalar_mul(
                            o_sb[:, 64 * a:64 * a + 64],
                            pv[:, 256 * a:256 * a + 64], rc[:, a:a + 1])
                    yield
                    pt = ppv.tile([128, 512], F32, name="pt", tag="pv")
                    ptv = pt[:, 0:64].bitcast(BF16)
                    nc.tensor.transpose(ptv, o_sb[:], id_t[:])
                    nc.vector.tensor_copy(
                        o_t[:, cc, 512 * j + qs:512 * j + qs + 128], ptv)
                    yield
                    if chase is not None and cc == 1:
                        for _ in chase:
                            break

        def gen_outproj(j, late=False):
            # late: the proj (ps) and scores (psc) pools are both idle --
            # cycle them so two psum pairs stay in flight.  st copies stay
            # on DVE while the exp stream still occupies ACT (an ACT copy
            # would queue behind the remaining exps, holding its psum slot);
            # only the final piece uses ACT, after the last exp has drained.
            # Non-late units yield between the two halves so a single ps
            # slot is held at a time (the proj chains share that pool).
            cyc = [(ps, "ps"), (psc, "sc")]
            for sl in range(4):
                sb = 4 * j + sl
                if late:
                    p0p, t0 = cyc[sl % 2]
                    p1p, t1 = cyc[(sl + 1) % 2]
                else:
                    (p0p, t0), (p1p, t1) = cyc[0], cyc[0]
                ps0 = p0p.tile([128, 512], F32, name="psC0", tag=t0)
                ps1 = p1p.tile([128, 512], F32, name="psC1", tag=t1)
                for k in range(2):
                    nc.tensor.matmul(ps0[:], o_t[:, k, bass.ts(sb, 128)],
                                     wo_t[:, k, 0:512],
                                     start=(k == 0), stop=(k == 1))
                for k in range(2):
                    nc.tensor.matmul(ps1[:], o_t[:, k, bass.ts(sb, 128)],
                                     wo_t[:, k, 512:1024],
                                     start=(k == 0), stop=(k == 1))
                st = stg.tile([128, 1024], BF16, name="st", tag="st")
                if late:
                    nc.scalar.activation(st[:, 0:512], ps0[:], AF.Copy)
                else:
                    nc.vector.tensor_copy(st[:, 0:512], ps0[:])
                nc.gpsimd.dma_start(outr[sb][:, 0:512], st[:, 0:512])
                nc.vector.tensor_copy(st[:, 512:1024], ps1[:])
                nc.sync.dma_start(outr[sb][:, 512:1024], st[:, 512:1024])
                yield

        def drain(g):
            for _ in g:
                pass

        def weave(primary, *others, stride=1):
            """Emit `primary` units round-robin with pieces from `others`."""
            gens = [g for g in others if g is not None]
            sts = list(stride) if isinstance(stride, (tuple, list)) \
                else [stride] * len(gens)
            n = 0
            while True:
                try:
                    next(primary)
                    n += 1
                except StopIteration:
                    break
                for g in list(gens):
                    st = sts[gens.index(g)]
                    reps = int(round(1 / st)) if st < 1 else \
                        (1 if n % st == 0 else 0)
                    for _ in range(reps):
                        try:
                            next(g)
                        except StopIteration:
                            idx = gens.index(g)
                            gens.pop(idx)
                            sts.pop(idx)
                            break
            for g in gens:
                for _ in g:
                    pass

        def chain(*gens):
            for g in gens:
                yield from g

        # software-pipelined emission.  PE executes in order, so nothing
        # that stalls on a late exp may precede ready work: pv(j) cc=0
        # follows scores(j) (its exps are done by then), while pv(j) cc=1
        # is deferred past scores(j+1) (its exps complete while scores(j+1)
        # runs).  outproj(0..2) fills quarter 3's exp-saturated window;
        # outproj(3) chases the cc=1 transposes inside pv(3).
        drain(gen_proj(0))
        weave(chain(gen_scores(0), gen_pv(0, ccs=(0,))), gen_proj(1))
        weave(chain(gen_scores(1), gen_pv(1, ccs=(0,))),
              chain(gen_pv(0, ccs=(1,)), gen_proj(2)), stride=2)
        weave(chain(gen_scores(2), gen_pv(2, ccs=(0,))),
              chain(gen_pv(1, ccs=(1,)), gen_proj(3), gen_outproj(0)),
              stride=1)
        weave(chain(gen_scores(3), gen_pv(3, ccs=(0,))),
              chain(gen_pv(2, ccs=(1,)), gen_outproj(1)), stride=3)
        drain(gen_outproj(2))
        drain(gen_pv(3, ccs=(1,), chase=gen_outproj(3, late=True)))

    nc.compile()
    return nc


_EO_IDX = None


def _eo_index():
    """Column permutation within one head group: all even components of the
    4 heads first (h-major), then all odd components."""
    global _EO_IDX
    if _EO_IDX is None:
        idx = []
        for eo in (0, 1):
            for h in range(HPG):
                idx.extend(range(64 * h + eo, 64 * h + 64, 2))
        _EO_IDX = np.asarray(idx)
    return _EO_IDX


def _split8(a):
    import ml_dtypes
    hi = np.asarray(a, dtype=np.float32).astype(ml_dtypes.float8_e4m3)
    lo = (np.asarray(a, dtype=np.float32) - hi.astype(np.float32)).astype(
        ml_dtypes.float8_e5m2)
    return np.ascontiguousarray(hi), np.ascontiguousarray(lo)


def make_in_maps(x, Wq, bq, Wk, bk, Wv, bv, Wo, bo, mask, freqs_cos, freqs_sin):
    import ml_dtypes
    idx = _eo_index()
    f32 = np.float32
    bf16 = ml_dtypes.bfloat16
    cosT = np.ascontiguousarray(freqs_cos.T, dtype=f32)       # (32, S)
    sinT = np.ascontiguousarray(freqs_sin.T, dtype=f32)
    cos4 = np.tile(cosT, (4, 1)).astype(bf16)                 # (128, S)
    sin4 = np.tile(sinT, (4, 1)).astype(bf16)
    # upper-triangular (incl. diagonal) ones: tri[k, q] = 1 iff k <= q
    tri = np.broadcast_to(np.triu(np.ones((128, 128), f32))[:, None, :],
                          (128, 2, 128)).copy().astype(bf16)
    idm = np.eye(128, dtype=f32).astype(bf16)

    in_maps = []
    for core in range(NCORES):
        b, g = core // G, core % G
        cols = slice(DG * g, DG * (g + 1))
        wq_g = np.ascontiguousarray(Wq[:, cols][:, idx], dtype=f32)
        wk_g = np.ascontiguousarray(Wk[:, cols][:, idx], dtype=f32)
        wv_g = np.ascontiguousarray(Wv[:, cols], dtype=f32)
        wqh_g, wql_g = _split8(wq_g)
        wkh_g, wkl_g = _split8(wk_g)
        wvh_g, wvl_g = _split8(wv_g)
        wo_g = np.ascontiguousarray(Wo[cols, :], dtype=f32).astype(bf16)
        bq_g = np.ascontiguousarray(
            bq[cols][idx].reshape(2, 128).T, dtype=f32)
        bk_g = np.ascontiguousarray(
            bk[cols][idx].reshape(2, 128).T, dtype=f32)
        xT_b = np.ascontiguousarray(np.asarray(x[b], dtype=f32).T)
        xh_b, xl_b = _split8(xT_b)
        in_maps.append(dict(xh=xh_b, xl=xl_b,
                            wqh=wqh_g, wql=wql_g, wkh=wkh_g, wkl=wkl_g,
                            wvh=wvh_g, wvl=wvl_g, wo=wo_g,
                            bqp=bq_g, bkp=bk_g, cos4=cos4, sin4=sin4,
                            trid=tri, idd=idm))
    return in_maps


_NC_CACHE = None
LAST_RESULTS = None


def kernel(**inputs):
    global _NC_CACHE
    if _NC_CACHE is None:
        _NC_CACHE = build_program()
    nc = _NC_CACHE

    inputs = {k: np.asarray(v) for k, v in inputs.items()}
    in_maps = make_in_maps(**inputs)
    kwargs = {}
    if os.environ.get("BASS_TRACE"):
        kwargs = dict(trace=True, trace_cores=list(range(NCORES)),
                      stitch_traces=True)
    res = run_bass_kernel_spmd(nc, in_maps, core_ids=list(range(NCORES)),
                               **kwargs)
    global LAST_RESULTS
    LAST_RESULTS = res

    out = np.zeros((B, S, HID), np.float32)
    for core in range(NCORES):
        out[core // G] += res.results[core]["out"].astype(np.float32)
    out += inputs["bo"].astype(np.float32)
    out += (inputs["bv"].astype(np.float32) @ inputs["Wo"].astype(np.float32))
    return out
